# revision 1
# baseline (speedup 1.0000x reference)
"""Chamfer-style point loss (nn_PointLoss) on 8 Trainium2 NeuronCores.

Math (reference): reflect points across plane n.x+d=0; half1 = reflected
points (valid where s=p.n+d < 0, mask m1), half2 = original points (mask
m2 = ~m1). D[i,j] = ||half1[i]-half2[j]||^2. Output scalar =
50*(sum_j min_i(D) m2_j / c2 + sum_i min_j(D) m1_i / c1).

v5 device formulation: the matmul computes -F directly (signs folded into
the A-side operands), F[i,j] = rr_a[i] + rr_b[j] + a_i.(-2 b_j) with
penalty P=2^14 on masked-out rows/cols. K-major operand images are built
on-chip with fp32 PE transposes of a [128, W, 16] composite (no DRAM
round trips, all APs at partition base 0 — offset bases and bf16
transposes silently produce zeros on HW): one [128,16]->[16,128]
transpose + PSUM->SBUF copy per row block (32) and per col block (4).
Main loop: one K=16 bf16 matmul per (128,512) tile; scalar bridges
PSUM->bf16; vector does row-max reduce (into the AllReduce payload) +
col-max accumulate. Column mins finish with 4 fp32 PE transposes + PSUM
reduces (no gpsimd custom ops, no DRAM staging). One AllReduce(max) on a
(128,33) f32 payload carries row mins + the s1 slot.

Sharding: half2 (column) axis split 8 ways, 512 cols/core; every core
holds all rows.
"""

import os
import sys

import numpy as np

for _p in ("/opt/trn_rl_repo", "/root/.axon_site/_ro/trn_rl_repo"):
    if os.path.isdir(_p) and _p not in sys.path:
        sys.path.insert(0, _p)

import concourse.bacc as bacc
import concourse.tile as tile
from concourse import mybir
from concourse.bass_utils import run_bass_kernel_spmd
from concourse.masks import make_identity

FP = mybir.dt.float32
BF = mybir.dt.bfloat16
AX = mybir.AxisListType
OP = mybir.AluOpType

N = 4096
NCORES = 8
QT = 32            # row q-slots ([p,q] is point 32p+q)
QC = QT // NCORES  # 4 col slots per partition (512 columns/core)
W = QT + QC        # merged row+col working width
NK = 16            # operand slots per block
PEN = float(2**14)
BIG = 1.0e30
CMINIT = -60000.0


def _emit(tc, out_ap, norm_ap, pa_ap, oh_ap):
    nc = tc.nc

    psf = tc.alloc_tile_pool(name="psf", bufs=3, space="PSUM")
    pst = tc.alloc_tile_pool(name="pst", bufs=3, space="PSUM")
    pss = tc.alloc_tile_pool(name="pss", bufs=1, space="PSUM")
    per = tc.alloc_tile_pool(name="per", bufs=1)
    fsp = tc.alloc_tile_pool(name="fsp", bufs=3)
    drm = tc.alloc_tile_pool(name="drm", bufs=1, space="DRAM")

    def _t(shape, name, dt=FP):
        return per.tile(shape, dt, name=name)

    # ---- constants / identities (gpsimd memsets, off critical path)
    IDEN = _t([128, 128], "IDEN")
    make_identity(nc, IDEN[:])

    # composite operand slots, fp32 (bf16 quantization already applied):
    # 0-2 hi(full W), 3-5 rows=hi cols=lo, 6-8 rows=lo cols=hi,
    # 9/10 rows=rr hi/lo (cols=1 memset), 11/12 cols=brr hi/lo
    # (rows=-1 memset), 13-15 lo(full W)
    SPL = _t([128, W, NK], "SPL")
    nc.gpsimd.memset(SPL[:, QT:W, 9:11], 1.0)
    nc.gpsimd.memset(SPL[:, 0:QT, 11:13], -1.0)

    ones_c = _t([128, 1], "ones_c")
    nc.gpsimd.memset(ones_c[:], 1.0)
    ones_r = _t([1, 128], "ones_r")
    nc.gpsimd.memset(ones_r[:], 1.0)

    # region-constant tiles (rows=first QT cols, cols=last QC)
    BETA = _t([128, W], "BETA")          # A: -1 * p   B: -2 * p
    nc.gpsimd.memset(BETA[:, 0:QT], -1.0)
    nc.gpsimd.memset(BETA[:, QT:W], -2.0)
    ROWP = _t([128, W], "ROWP")          # penalty offset: rows -PEN, cols 0
    nc.gpsimd.memset(ROWP[:, 0:QT], -PEN)
    nc.gpsimd.memset(ROWP[:, QT:W], 0.0)
    RMSK = _t([128, W], "RMSK")          # +1 rows, 0 cols
    nc.gpsimd.memset(RMSK[:, 0:QT], 1.0)
    nc.gpsimd.memset(RMSK[:, QT:W], 0.0)
    SGN1 = _t([128, W], "SGN1")          # -1 rows, +1 cols (pp sign)
    nc.gpsimd.memset(SGN1[:, 0:QT], -1.0)
    nc.gpsimd.memset(SGN1[:, QT:W], 1.0)
    CM = _t([128, 512], "CM", BF)
    nc.gpsimd.memset(CM[:], CMINIT)

    # ---- inputs to SBUF
    norm_sb = _t([1, 4], "norm_sb")
    nc.sync.dma_start(norm_sb[:], norm_ap[:])
    PA = _t([128, W, 3], "PA")
    nc.sync.dma_start(PA[:], pa_ap[:])
    OH = _t([128, 1], "OH")
    nc.scalar.dma_start(OH[:], oh_ap[:])

    # ---- norm broadcast + plane constants
    NB_ps = pss.tile([128, 4], FP, tag="bc")
    nc.tensor.matmul(NB_ps[:], ones_r[:], norm_sb[:], start=True, stop=True)
    NB = _t([128, 4], "NB")
    nc.scalar.copy(NB[:], NB_ps[:])
    nsq = _t([128, 4], "nsq")
    nc.vector.tensor_tensor(nsq[:], NB[:], NB[:], op=OP.mult)
    snn = _t([128, 1], "snn")
    nc.vector.tensor_reduce(snn[:], nsq[:, 0:3], axis=AX.X, op=OP.add)
    inv_nn = _t([128, 1], "inv_nn")
    nc.vector.reciprocal(inv_nn[:], snn[:])
    pinv2 = _t([128, 1], "pinv2")       # +2/nn  (negated-A alpha)
    nc.scalar.mul(pinv2[:], inv_nn[:], 2.0)
    c4d = _t([128, 1], "c4d")           # -4d/nn (negated-A t3)
    nc.vector.tensor_tensor(c4d[:], NB[:, 3:4], inv_nn[:], op=OP.mult)
    nc.scalar.mul(c4d[:], c4d[:], -4.0)
    PINV2R = _t([128, W], "PINV2R")
    nc.vector.tensor_scalar(PINV2R[:], RMSK[:], pinv2[:], None, op0=OP.mult)
    C4DR = _t([128, W], "C4DR")
    nc.scalar.mul(C4DR[:], RMSK[:], c4d[:])

    # ---- merged plane eval: s = p.n + d, m1 = (s<0)
    s_all = _t([128, W], "s_all")
    t1_ = _t([128, W], "t1_")
    nc.scalar.mul(s_all[:], PA[:, :, 0], NB[:, 0:1])
    nc.scalar.mul(t1_[:], PA[:, :, 1], NB[:, 1:2])
    nc.vector.tensor_tensor(s_all[:], s_all[:], t1_[:], op=OP.add)
    nc.scalar.mul(t1_[:], PA[:, :, 2], NB[:, 2:3])
    nc.vector.tensor_tensor(s_all[:], s_all[:], t1_[:], op=OP.add)
    nc.vector.tensor_scalar_add(s_all[:], s_all[:], NB[:, 3:4])
    M1 = _t([128, W], "M1")
    nc.vector.tensor_scalar(M1[:], s_all[:], 0.0, None, op0=OP.is_lt)

    # ---- operand vectors V = alpha*n + beta*p (A rows pre-negated)
    alpha = _t([128, W], "alpha")
    nc.vector.tensor_tensor(alpha[:], s_all[:], PINV2R[:], op=OP.mult)
    V = []
    for c in range(3):
        tv = _t([128, W], f"tv{c}")
        nc.scalar.mul(tv[:], alpha[:], NB[:, c : c + 1])
        tb2 = _t([128, W], f"tb2{c}")
        nc.vector.tensor_tensor(tb2[:], BETA[:], PA[:, :, c], op=OP.mult)
        v = _t([128, W], f"v{c}")
        nc.vector.tensor_tensor(v[:], tv[:], tb2[:], op=OP.add)
        V.append(v)

    # ---- rr' = sgn*(|p|^2) + s*C4DR + (M1*PEN + ROWP)
    pp = _t([128, W], "pp")
    nc.vector.tensor_tensor(pp[:], PA[:, :, 0], PA[:, :, 0], op=OP.mult)
    q1 = _t([128, W], "q1")
    nc.vector.tensor_tensor(q1[:], PA[:, :, 1], PA[:, :, 1], op=OP.mult)
    nc.vector.tensor_tensor(pp[:], pp[:], q1[:], op=OP.add)
    q2 = _t([128, W], "q2")
    nc.vector.tensor_tensor(q2[:], PA[:, :, 2], PA[:, :, 2], op=OP.mult)
    nc.vector.tensor_tensor(pp[:], pp[:], q2[:], op=OP.add)
    nc.vector.tensor_tensor(pp[:], pp[:], SGN1[:], op=OP.mult)
    t3 = _t([128, W], "t3")
    nc.vector.tensor_tensor(t3[:], s_all[:], C4DR[:], op=OP.mult)
    t4 = _t([128, W], "t4")
    nc.vector.tensor_scalar(t4[:], M1[:], PEN, None, op0=OP.mult)
    nc.vector.tensor_tensor(t4[:], t4[:], ROWP[:], op=OP.add)
    rr = _t([128, W], "rr")
    nc.vector.tensor_tensor(rr[:], pp[:], t3[:], op=OP.add)
    nc.vector.tensor_tensor(rr[:], rr[:], t4[:], op=OP.add)

    # ---- bf16 hi/lo splits into the fp32 composite
    for c in range(3):
        bsc = _t([128, W], f"bsc{c}", BF)
        nc.scalar.copy(bsc[:], V[c][:])
        nc.vector.tensor_scalar(SPL[:, :, c], bsc[:], 1.0, None, op0=OP.mult)
        nc.vector.tensor_tensor(
            SPL[:, :, 13 + c], V[c][:], SPL[:, :, c], op=OP.subtract
        )
        nc.scalar.copy(SPL[:, 0:QT, 3 + c], SPL[:, 0:QT, c])
        nc.vector.tensor_scalar(
            SPL[:, QT:W, 3 + c], SPL[:, QT:W, 13 + c], 1.0, None, op0=OP.mult
        )
        nc.vector.tensor_scalar(
            SPL[:, 0:QT, 6 + c], SPL[:, 0:QT, 13 + c], 1.0, None, op0=OP.mult
        )
        nc.scalar.copy(SPL[:, QT:W, 6 + c], SPL[:, QT:W, c])

    bsr = _t([128, W], "bsr", BF)
    nc.scalar.copy(bsr[:], rr[:])
    hfr = _t([128, W], "hfr")
    nc.vector.tensor_scalar(hfr[:], bsr[:], 1.0, None, op0=OP.mult)
    lrr = _t([128, W], "lrr")
    nc.vector.tensor_tensor(lrr[:], rr[:], hfr[:], op=OP.subtract)
    nc.scalar.copy(SPL[:, 0:QT, 9], hfr[:, 0:QT])
    nc.scalar.copy(SPL[:, QT:W, 11], hfr[:, QT:W])
    nc.scalar.copy(SPL[:, 0:QT, 10], lrr[:, 0:QT])
    nc.scalar.copy(SPL[:, QT:W, 12], lrr[:, QT:W])

    # ---- K-major operands via fp32 PE transposes (all base partition 0)
    TB = _t([16, 512], "TB", BF)
    for c in range(QC):
        tcp = pst.tile([128, 128], FP, name="tp")
        nc.tensor.transpose(tcp[0:16, :], SPL[:, QT + c, :], IDEN[:])
        if c % 2 == 0:
            nc.scalar.copy(TB[:, 128 * c : 128 * (c + 1)], tcp[0:16, :])
        else:
            nc.vector.tensor_scalar(
                TB[:, 128 * c : 128 * (c + 1)], tcp[0:16, :], 1.0, None, op0=OP.mult
            )

    TAS = _t([16, QT * 128], "TAS", BF)
    for m in range(QT):
        ttp = pst.tile([128, 128], FP, name="tp")
        nc.tensor.transpose(ttp[0:16, :], SPL[:, m, :], IDEN[:])
        if m % 2 == 0:
            nc.scalar.copy(TAS[:, 128 * m : 128 * (m + 1)], ttp[0:16, :])
        else:
            nc.vector.tensor_scalar(
                TAS[:, 128 * m : 128 * (m + 1)], ttp[0:16, :], 1.0, None, op0=OP.mult
            )

    # ---- c1/c2 + reciprocals precomputed before the collective
    c1row = _t([128, 1], "c1row")
    nc.vector.tensor_reduce(c1row[:], M1[:, 0:QT], axis=AX.X, op=OP.add)
    c1_ps = pss.tile([1, 1], FP, tag="ps")
    nc.tensor.matmul(c1_ps[:], c1row[:], ones_c[:], start=True, stop=True)
    c1 = _t([1, 1], "c1")
    nc.scalar.copy(c1[:], c1_ps[:])
    c2 = _t([1, 1], "c2")
    nc.vector.tensor_scalar(c2[:], c1[:], -1.0, float(N), op0=OP.mult, op1=OP.add)
    nc.vector.tensor_scalar_max(c1[:], c1[:], 1.0)
    nc.vector.tensor_scalar_max(c2[:], c2[:], 1.0)
    rc1 = _t([1, 1], "rc1")
    nc.vector.reciprocal(rc1[:], c1[:])
    rc2 = _t([1, 1], "rc2")
    nc.vector.reciprocal(rc2[:], c2[:])

    M2CB = _t([128, QC], "M2CB")
    nc.vector.tensor_scalar(M2CB[:], M1[:, QT:W], -1.0, 1.0, op0=OP.mult, op1=OP.add)
    bm = _t([128, 1], "bm")
    nc.vector.tensor_scalar(bm[:], OH[:], BIG, -BIG, op0=OP.mult, op1=OP.add)

    # ---- main loop: PSUM gets -F; row maxes into payload; col-max acc
    PAYSB = _t([128, QT + 1], "PAYSB")

    for m in range(QT):
        fps = psf.tile([128, 512], FP, name="fps")
        nc.tensor.matmul(
            fps[:],
            TAS[:, 128 * m : 128 * (m + 1)],
            TB[:],
            start=True,
            stop=True,
        )
        FS = fsp.tile([128, 512], BF, name="FS")
        nc.scalar.copy(FS[:], fps[:])
        nc.vector.tensor_reduce(PAYSB[:, m : m + 1], FS[:], axis=AX.X, op=OP.max)
        nc.vector.tensor_tensor(CM[:], CM[:], FS[:], op=OP.max)

    # ---- columns: d1 = max over partitions via fp32 PE transposes
    CMF = _t([128, 512], "CMF")
    nc.scalar.copy(CMF[:], CM[:])
    d1t = _t([128, QC], "d1t")
    for h in range(4):
        tdp = pst.tile([128, 128], FP, name="tp")
        nc.tensor.transpose(tdp[:], CMF[:, 128 * h : 128 * (h + 1)], IDEN[:])
        nc.vector.tensor_reduce(d1t[:, h : h + 1], tdp[:], axis=AX.X, op=OP.max)

    # s1 = sum(d1t * m2) ; encode into one-hot slot column
    w1j = _t([128, QC], "w1j")
    vsum = _t([128, 1], "vsum")
    nc.vector.tensor_tensor(w1j[:], d1t[:], M2CB[:], op=OP.mult)
    nc.vector.tensor_reduce(vsum[:], w1j[:], axis=AX.X, op=OP.add)
    s1_ps = pss.tile([1, 1], FP, tag="ps")
    nc.tensor.matmul(s1_ps[:], vsum[:], ones_c[:], start=True, stop=True)
    s1sb = _t([1, 1], "s1sb")
    nc.scalar.copy(s1sb[:], s1_ps[:])
    bc_ps = pss.tile([128, 4], FP, tag="bc")
    nc.tensor.matmul(bc_ps[:, 0:1], ones_r[:], s1sb[:], start=True, stop=True)
    slotv = _t([128, 1], "slotv")
    nc.vector.tensor_tensor(slotv[:], bc_ps[:, 0:1], OH[:], op=OP.mult)
    nc.vector.tensor_tensor(PAYSB[:, QT : QT + 1], slotv[:], bm[:], op=OP.add)

    # ---- AllReduce(max) of [D2 | slot] over all 8 cores
    pay = drm.tile([128, QT + 1], FP, name="pay")
    pay2 = drm.tile([128, QT + 1], FP, name="pay2")
    nc.sync.dma_start(pay[:, 0:QT], PAYSB[:, 0:QT])
    nc.scalar.dma_start(pay[:, QT : QT + 1], PAYSB[:, QT : QT + 1])
    nc.gpsimd.collective_compute(
        "AllReduce",
        OP.max,
        replica_groups=[list(range(NCORES))],
        ins=[pay.opt()],
        outs=[pay2.opt()],
    )
    RB = _t([128, QT + 1], "RB")
    nc.scalar.dma_start(RB[:], pay2[:])

    # ---- finish: s2 = sum(G2*m1), sum slots, combine with rc1/rc2
    w2j = _t([128, QT], "w2j")
    w2s = _t([128, 1], "w2s")
    nc.vector.tensor_tensor(w2j[:], RB[:, 0:QT], M1[:, 0:QT], op=OP.mult)
    nc.vector.tensor_reduce(w2s[:], w2j[:], axis=AX.X, op=OP.add)
    s2_ps = pss.tile([1, 1], FP, tag="ps")
    nc.tensor.matmul(s2_ps[:], w2s[:], ones_c[:], start=True, stop=True)
    sa_ps = pss.tile([1, 1], FP, tag="ps")
    nc.tensor.matmul(
        sa_ps[:], RB[0:NCORES, QT : QT + 1], ones_c[0:NCORES, :], start=True, stop=True
    )

    av2 = _t([1, 1], "av2")
    nc.vector.tensor_tensor(av2[:], s2_ps[:], rc1[:], op=OP.mult)
    av1 = _t([1, 1], "av1")
    nc.vector.tensor_tensor(av1[:], sa_ps[:], rc2[:], op=OP.mult)
    res = _t([1, 1], "res")
    nc.vector.tensor_tensor(res[:], av1[:], av2[:], op=OP.add)
    nc.scalar.mul(res[:], res[:], -50.0)
    nc.sync.dma_start(out_ap[:], res[:])

    for p in (psf, pst, pss, per, fsp, drm):
        p.seal()


_NC = None


def build():
    global _NC
    if _NC is not None:
        return _NC
    nc = bacc.Bacc(
        "TRN2", target_bir_lowering=False, debug=False, num_devices=NCORES
    )
    norm_ap = nc.dram_tensor("norm4", [1, 4], FP, kind="ExternalInput").ap()
    pa_ap = nc.dram_tensor("pa", [128, W, 3], FP, kind="ExternalInput").ap()
    oh_ap = nc.dram_tensor("oh", [128, 1], FP, kind="ExternalInput").ap()
    out_ap = nc.dram_tensor("out", [1, 1], FP, kind="ExternalOutput").ap()
    with tile.TileContext(nc) as tc:
        _emit(tc, out_ap, norm_ap, pa_ap, oh_ap)
    nc.compile()
    _NC = nc
    return nc


def make_in_maps(norm, points):
    norm = np.ascontiguousarray(norm, dtype=np.float32)
    pts = np.ascontiguousarray(points, dtype=np.float32)
    PTq = pts.reshape(128, QT, 3)
    maps = []
    for c in range(NCORES):
        oh = np.zeros((128, 1), np.float32)
        oh[c, 0] = 1.0
        cb = pts[512 * c : 512 * (c + 1)].reshape(128, QC, 3)
        pa = np.ascontiguousarray(np.concatenate([PTq, cb], axis=1))
        maps.append({"norm4": norm, "pa": pa, "oh": oh})
    return maps


LAST_RESULTS = None


def kernel(norm, points):
    global LAST_RESULTS
    nc = build()
    maps = make_in_maps(norm, points)
    trace = bool(os.environ.get("KERNEL_TRACE"))
    LAST_RESULTS = run_bass_kernel_spmd(
        nc, maps, list(range(NCORES)), trace=trace
    )
    out = np.asarray(LAST_RESULTS.results[0]["out"], dtype=np.float32)
    return out.reshape(())

